# revision 8
# baseline (speedup 1.0000x reference)
"""Trainium2 Bass kernel for nn_CharRNN: 2-layer LSTM LM + big softmax matmul.

Strategy (8 NeuronCores, SPMD, no collectives):
  - Every core runs the full-batch LSTM recurrence (replicated; the recurrent
    matmul streaming cost is batch-independent, so sharding it buys nothing
    and would force per-step communication).
  - The [B*T, V] logits matmul is sharded over the vocab axis: core k owns
    softmax_w[:, 4000k:4000(k+1)] and writes its 65 MB logits slice.
  - Each core also emits per-token sum(exp(logits_shard)) so the CE loss is
    finished on the host with an 8-way log-sum-exp combine (no device comm).

Device layout notes:
  - Tokens are kept t-major on device: column index = t*128 + k*32 + b for the
    transposed activations (k = 128-row chunk of H).  The logits DMA writes
    them back in b-major order so the returned array matches the reference.
  - The per-step gate matmul uses PE column tiling: col-group g computes gate
    g (host permutes gates to (i,f,o,j)) so the [128,512] gate PSUM tile has
    batch+gate on partitions and hidden on free, giving full-width ACT/DVE
    elementwise work.
  - All matmuls run as float32r (TF32-like single-pass fp32: 1 cycle/row).
"""

import numpy as np
import ml_dtypes

V, B, T, H = 32000, 32, 128, 512
NC = 8
VS = V // NC            # vocab shard per core = 4000
NB = 8                  # N-chunks per vocab shard
NSZ = VS // NB          # 500 columns per matmul (<= 512 fp32 PSUM bank)
MT = (B * T) // 128     # 32 token M-tiles

_NC_CACHE = {}
_DEBUG_H2_FLAG = [False]


def _build(has_lstm_bias):
    import concourse.bacc as bacc
    import concourse.mybir as mybir
    import concourse.tile as tile
    from concourse.masks import make_identity

    F32 = mybir.dt.float32
    F32R = mybir.dt.float32r
    BF16 = mybir.dt.bfloat16
    AF = mybir.ActivationFunctionType

    nc = bacc.Bacc("TRN2", target_bir_lowering=False, debug=False, num_devices=NC)

    xTp = nc.dram_tensor("xTp", [128, T * 128], BF16, kind="ExternalInput")
    w1 = nc.dram_tensor("w1", [1024, 2048], BF16, kind="ExternalInput")
    w2 = nc.dram_tensor("w2", [1024, 2048], BF16, kind="ExternalInput")
    smw = nc.dram_tensor("smw", [512, VS], F32R, kind="ExternalInput")
    smb = nc.dram_tensor("smb", [128, VS], F32, kind="ExternalInput")
    bl = None
    if has_lstm_bias:
        bl = [
            nc.dram_tensor(f"bl{l}", [128, 512], F32, kind="ExternalInput")
            for l in range(2)
        ]
    logits_o = nc.dram_tensor("logits_o", [B, T, VS], F32, kind="ExternalOutput")
    h2dump = None
    if globals().get("_DEBUG_H2", False) or _DEBUG_H2_FLAG[0]:
        h2dump = nc.dram_tensor("h2dump", [128, T * 128], F32R, kind="ExternalOutput")
    sumexp_o = nc.dram_tensor("sumexp_o", [128, MT], F32, kind="ExternalOutput")

    with tile.TileContext(nc) as tc:
        with (
            tc.tile_pool(name="constp", bufs=1) as constp,
            tc.tile_pool(name="dramp", bufs=1, space="DRAM") as dramp,
        ):
            ident = constp.tile([128, 128], F32)
            make_identity(nc, ident[:])
            h2T_d = dramp.tile([128, T * 128], F32R)

            # ---------------- LSTM recurrence ----------------
            with (
                tc.tile_pool(name="wp", bufs=1) as wp,
                tc.tile_pool(name="xtp", bufs=4) as xtp,
                tc.tile_pool(name="state", bufs=3) as statep,
                tc.tile_pool(name="elt", bufs=3) as eltp,
                tc.tile_pool(name="pgp", bufs=4, space="PSUM") as pgp,
                tc.tile_pool(name="ptp", bufs=2, space="PSUM") as ptp,
            ):
                w_sb = []
                for l, wd in enumerate((w1, w2)):
                    wt = wp.tile([128, 8 * 2048], BF16, tag=f"w{l}")
                    for kc in range(8):
                        nc.sync.dma_start(
                            wt[:, kc * 2048 : (kc + 1) * 2048],
                            wd[kc * 128 : (kc + 1) * 128, :],
                        )
                    w_sb.append(wt)
                blt = []
                if has_lstm_bias:
                    for l in range(2):
                        bt_ = wp.tile([128, 512], F32, tag=f"bl{l}")
                        nc.sync.dma_start(bt_[:], bl[l][:])
                        blt.append(bt_)

                hT_prev, c_prev = [], []
                for l in range(2):
                    ht0 = statep.tile([128, 128], BF16, tag=f"hT{l}")
                    nc.vector.memset(ht0[:].bitcast(mybir.dt.uint16), 0)
                    c0 = statep.tile([64, 512], F32, tag=f"c{l}")
                    nc.vector.memset(c0[32:64, :], 0.0)
                    hT_prev.append(ht0)
                    c_prev.append(c0)

                for t in range(T):
                    xt = xtp.tile([128, 128], BF16, tag="xt")
                    nc.sync.dma_start(xt[:], xTp[:, t * 128 : (t + 1) * 128])
                    stat_x = xt
                    for l in range(2):
                        pg = pgp.tile([128, 512], F32, tag="pg")
                        for k in range(8):
                            stat = stat_x if k < 4 else hT_prev[l]
                            kk = k % 4
                            lhsT = stat[:, 32 * kk : 32 * (kk + 1)]
                            for g in range(4):
                                rhs = w_sb[l][
                                    :, k * 2048 + 512 * g : k * 2048 + 512 * (g + 1)
                                ]
                                nc.tensor.matmul(
                                    pg[32 * g : 32 * (g + 1), :],
                                    lhsT,
                                    rhs,
                                    start=(k == 0),
                                    stop=(k == 7),
                                    tile_position=(0, 32 * g),
                                )
                        if has_lstm_bias:
                            nc.vector.tensor_add(pg[:], pg[:], blt[l][:])
                        # gates layout: partition 32g+b, free = hidden n
                        # g: 0=i 1=f 2=o (sigmoid), 3=j (tanh)
                        # base-partition alignment: DVE tensor_tensor needs
                        # equal base partitions for both SBUF inputs.
                        # sig: i@0:32 f@32:64 o@64:96; c lives at rows 32:64;
                        # tanh(c) at rows 64:96.
                        sig = eltp.tile([96, 512], F32, tag="sig")
                        nc.scalar.activation(sig[:], pg[0:96, :], AF.Sigmoid)
                        tj = eltp.tile([32, 512], F32, tag="tj")
                        nc.scalar.activation(tj[:], pg[96:128, :], AF.Tanh)
                        tmp1 = eltp.tile([32, 512], F32, tag="tmp1")
                        nc.vector.tensor_mul(tmp1[:], c_prev[l][32:64, :], sig[32:64, :])
                        tmp2 = eltp.tile([32, 512], F32, tag="tmp2")
                        nc.vector.tensor_mul(tmp2[:], sig[0:32, :], tj[:])
                        c_new = statep.tile([64, 512], F32, tag=f"c{l}")
                        nc.vector.tensor_add(c_new[32:64, :], tmp1[:], tmp2[:])
                        tc_t = eltp.tile([96, 512], F32, tag="tc")
                        nc.scalar.activation(tc_t[64:96, :], c_new[32:64, :], AF.Tanh)
                        h = eltp.tile([32, 512], F32, tag="h")
                        nc.vector.tensor_mul(h[:], tc_t[64:96, :], sig[64:96, :])
                        # h [32, 512] -> hT [128, 4*32] (chunk k at cols 32k)
                        pt = ptp.tile([128, 128], F32, tag="pt")
                        for k in range(4):
                            nc.tensor.transpose(
                                pt[:, 32 * k : 32 * (k + 1)],
                                h[:, 128 * k : 128 * (k + 1)],
                                ident[0:32, 0:32],
                            )
                        hT = statep.tile([128, 128], BF16, tag=f"hT{l}")
                        nc.vector.tensor_copy(hT[:], pt[:])
                        if l == 1:
                            hT_fr = statep.tile([128, 128], F32R, tag="hTfr")
                            nc.vector.tensor_copy(hT_fr[:], pt[:])
                        c_prev[l] = c_new
                        hT_prev[l] = hT
                        stat_x = hT  # layer 2 x-input = layer 1 h
                    # h2T stored k-major: column = k*4096 + t*32 + b
                    h2v = h2T_d[:].rearrange("p (k tok) -> p k tok", k=4)
                    nc.sync.dma_start(
                        h2v[:, :, 32 * t : 32 * (t + 1)],
                        hT_fr[:].rearrange("p (k b) -> p k b", k=4),
                    )

            # ---------------- logits + softmax stats ----------------
            with (
                tc.tile_pool(name="smwp", bufs=1) as smwp,
                tc.tile_pool(name="h2tp", bufs=3) as h2tp,
                tc.tile_pool(name="ltp", bufs=2) as ltp,
                tc.tile_pool(name="escrp", bufs=2) as escrp,
                tc.tile_pool(name="sep", bufs=1) as sep,
                tc.tile_pool(name="plp", bufs=4, space="PSUM") as plp,
            ):
                smw_sb = smwp.tile([128, 4 * VS], F32R, tag="smw")
                for k in range(4):
                    nc.sync.dma_start(
                        smw_sb[:, k * VS : (k + 1) * VS],
                        smw[k * 128 : (k + 1) * 128, :],
                    )
                smb_sb = smwp.tile([128, VS], F32, tag="smb")
                nc.sync.dma_start(smb_sb[:], smb[:])
                se_sb = sep.tile([128, MT], F32)
                for m in range(MT):
                    h2t = h2tp.tile([128, 512], F32R, tag="h2t")
                    h2vv = h2T_d[:].rearrange("p (k tok) -> p k tok", k=4)
                    nc.sync.dma_start(
                        h2t[:].rearrange("p (k c) -> p k c", k=4),
                        h2vv[:, :, 128 * m : 128 * (m + 1)],
                    )
                    lt = ltp.tile([128, VS], F32, tag="lt")
                    for nb in range(NB):
                        pl = plp.tile([128, NSZ], F32, tag="pl")
                        for k in range(4):
                            lhsT = h2t[:, 128 * k : 128 * (k + 1)]
                            rhs = smw_sb[
                                :, k * VS + nb * NSZ : k * VS + (nb + 1) * NSZ
                            ]
                            nc.tensor.matmul(
                                pl[:], lhsT, rhs, start=(k == 0), stop=(k == 3)
                            )
                        nc.vector.tensor_add(
                            lt[:, nb * NSZ : (nb + 1) * NSZ],
                            pl[:],
                            smb_sb[:, nb * NSZ : (nb + 1) * NSZ],
                        )
                    escr = escrp.tile([128, VS], F32, tag="escr")
                    nc.scalar.activation(
                        escr[:], lt[:], AF.Exp, accum_out=se_sb[:, m : m + 1]
                    )
                    for dt in range(4):
                        nc.sync.dma_start(
                            logits_o[:, 4 * m + dt, :],
                            lt[32 * dt : 32 * (dt + 1), :],
                        )
                nc.sync.dma_start(sumexp_o[:], se_sb[:])
                if h2dump is not None:
                    nc.sync.dma_start(h2dump[:], h2T_d[:])

    nc.compile()
    return nc


def _get_nc(has_lstm_bias):
    key = bool(has_lstm_bias)
    if key not in _NC_CACHE:
        _NC_CACHE[key] = _build(key)
    return _NC_CACHE[key]


# gate permutation: TF order (i, j, f, o) -> device order (i, f, o, j)
_GATE_PERM = np.concatenate(
    [
        np.arange(0, 512),
        np.arange(1024, 1536),
        np.arange(1536, 2048),
        np.arange(512, 1024),
    ]
)


def kernel(**inputs):
    ids = np.asarray(inputs["input_data"]).astype(np.int64)
    tgt = np.asarray(inputs["targets"]).astype(np.int64)
    emb = np.asarray(inputs["embedding"], dtype=np.float32)
    lw = np.asarray(inputs["lstm_w"], dtype=np.float32)
    lb = np.asarray(inputs["lstm_b"], dtype=np.float32)
    smw = np.asarray(inputs["softmax_w"], dtype=np.float32)
    smb = np.asarray(inputs["softmax_b"], dtype=np.float32)

    # host embedding gather + transpose to device layout
    x = emb[ids]  # [B, T, H]
    m_ = np.moveaxis(x, 2, 0).reshape(4, 128, B, T)  # [k, r, b, t]
    xTp = (
        np.ascontiguousarray(m_.transpose(1, 3, 0, 2))
        .reshape(128, T * 128)
        .astype(ml_dtypes.bfloat16)
    )

    wps = [
        np.ascontiguousarray(lw[l][:, _GATE_PERM]).astype(ml_dtypes.bfloat16)
        for l in range(2)
    ]
    has_b = bool(np.any(lb))

    in_maps = []
    for c in range(NC):
        im = {
            "xTp": xTp,
            "w1": wps[0],
            "w2": wps[1],
            "smw": np.ascontiguousarray(smw[:, c * VS : (c + 1) * VS]),
            "smb": np.ascontiguousarray(
                np.broadcast_to(smb[c * VS : (c + 1) * VS], (128, VS))
            ),
        }
        if has_b:
            for l in range(2):
                bp = lb[l][_GATE_PERM].reshape(4, 1, 512)
                im[f"bl{l}"] = np.ascontiguousarray(
                    np.repeat(bp, 32, axis=1)
                ).reshape(128, 512)
        in_maps.append(im)

    from concourse.bass_utils import run_bass_kernel_spmd

    nc = _get_nc(has_b)
    res = run_bass_kernel_spmd(nc, in_maps, core_ids=list(range(NC)))

    logits_full = np.empty((B * T, V), np.float32)
    se = np.zeros((T, B), np.float64)
    for c in range(NC):
        logits_full[:, c * VS : (c + 1) * VS] = res.results[c]["logits_o"].reshape(
            B * T, VS
        )
        s = res.results[c]["sumexp_o"]  # [128, MT], partition = dt*32 + b
        se += s.reshape(4, B, MT).transpose(2, 0, 1).reshape(T, B)

    lse_flat = np.log(se).T.reshape(-1)  # [B*T] b-major
    tflat = tgt.reshape(-1)
    tlogit = logits_full[np.arange(B * T), tflat].astype(np.float64)
    cost = np.float32((lse_flat - tlogit).mean())
    return logits_full, cost


if __name__ == "__main__":
    rng = np.random.default_rng(0)
    demo = {
        "input_data": rng.integers(0, V, (B, T)),
        "targets": rng.integers(0, V, (B, T)),
        "embedding": rng.uniform(-1, 1, (V, H)).astype(np.float32),
        "lstm_w": (rng.standard_normal((2, 2 * H, 4 * H)) * 0.05).astype(np.float32),
        "lstm_b": np.zeros((2, 4 * H), np.float32),
        "softmax_w": (rng.standard_normal((H, V)) / np.sqrt(H)).astype(np.float32),
        "softmax_b": np.zeros((V,), np.float32),
    }
    lg, c = kernel(**demo)
    print(lg.shape, lg.dtype, c)
